# revision 12
# baseline (speedup 1.0000x reference)
"""GAT layer kernel for 8 Trainium2 NeuronCores — v3.

Edge-parallel dense-rank strategy (dst-sharded) as v2, restructured for
engine balance (v2 was DVE- and phase-B-serialization-bound):

  - pair gathers unchanged (512B elements, GQ=1024, NQ=2 — measured
    optimal at 0.87 ns/idx on HW; 256B elements run at half the rate).
  - alpha math batched over superchunks of 8 chunks (64 slots wide) on
    [128, 64] tiles: ~10 medium DVE ops per 8192 edges instead of ~10
    small ops per 1024 edges. lrelu fused via scalar_tensor_tensor.
  - the hi/lo pair select is folded into the alpha weighting:
    contribution = axl*lo + axh*hi with axl = ax*(1-par), axh = ax*par.
    Two big DVE muls per chunk; no 3-op select chain.
  - accumulation into PSUM-resident [128, 7, 7*65] f32 (7 windows x 7
    blocks x 65 cols = all 49 dst blocks live in PSUM), via identity-lhsT
    matmuls over maximal block-runs; alpha_sum rides along as rhs col 64.
  - phase B: fp16 x input (half the read bytes), 4-block PSUM tiles,
    2 ACT copies per 4 blocks, XB=16 batches, dedicated pools.
  - s_dst per position from a host-permuted fp16 x copy via 49 one-col
    matmuls (replaces v2's phase-B2 table gathers).
"""
import os
import sys
import numpy as np

_REP = int(os.environ.get("KGAT_REPEAT", "1"))
_ABL = set(os.environ.get("KGAT_ABLATE", "").split(","))

try:
    import concourse.bacc as bacc
except ImportError:
    sys.path.insert(0, "/opt/trn_rl_repo")
    import concourse.bacc as bacc
import concourse.tile as tile
import concourse.mybir as mybir
from concourse import bass_utils
from concourse.masks import make_identity

C = 8
GQ = int(os.environ.get("KGAT_GQ", "1024"))
NQ = int(os.environ.get("KGAT_NQ", "2"))
SCR = int(os.environ.get("KGAT_SCR", "36864"))
SK = int(os.environ.get("KGAT_SK", "8"))      # chunks per superchunk
WIN = 7                                        # blocks per PSUM window
TROW = 128            # fp16 per table row (256B)
PAIR = 2 * TROW       # fp16 per gather element (512B)
XB = 16               # phase-B nodes per batch / 128

F32 = mybir.dt.float32
F16 = mybir.dt.float16
I16 = mybir.dt.int16
ALU = mybir.AluOpType
ACTF = mybir.ActivationFunctionType


def _sig(n):
    """Node -> table row: within each 256-group interleave halves so that
    phase B writes row pairs (2p, 2p+1) contiguously from partition p."""
    return (n // 256) * 256 + 2 * (n % 128) + (n % 256) // 128


def _wrap16(a):
    w = np.ascontiguousarray(a.reshape(-1, 16).T)
    return np.tile(w, (8, 1))


def _prep(edge_index, edge_weight, N):
    NL = N // C
    NB = -(-NL // 128)
    NLP = NB * 128
    src = np.asarray(edge_index[0], dtype=np.int64)
    dst = np.asarray(edge_index[1], dtype=np.int64)
    w = np.asarray(edge_weight, dtype=np.float32)

    cores = []
    max_cnt = np.zeros(0, np.int64)
    for c in range(C):
        m = (dst >= c * NL) & (dst < (c + 1) * NL)
        s_c = src[m]
        d_c = dst[m] - c * NL
        w_c = w[m]
        deg = np.bincount(d_c, minlength=NL)
        perm = np.argsort(-deg, kind="stable")
        order = np.argsort(d_c, kind="stable")
        starts = np.zeros(NL + 1, np.int64)
        starts[1:] = np.cumsum(deg)
        maxdeg = int(deg.max()) if deg.size else 0
        hist = np.bincount(deg, minlength=maxdeg + 2)
        cnt = NL - np.cumsum(hist)[:maxdeg + 1]
        cnt = cnt[cnt > 0]
        cores.append(dict(s=s_c, w=w_c, perm=perm, order=order,
                          starts=starts, cnt=cnt))
        if len(cnt) > len(max_cnt):
            mc = np.zeros(len(cnt), np.int64)
            mc[:len(max_cnt)] = max_cnt
            max_cnt = mc
        max_cnt[:len(cnt)] = np.maximum(max_cnt[:len(cnt)], cnt)

    # rank 0 must cover every position block so its runs span full PSUM
    # windows (start=True resets whole windows)
    max_cnt[0] = NLP

    # unified slot stream: rank-major, block = slot index within rank
    nb_r = [int(-(-int(x) // 128)) for x in max_cnt]
    blocks = []
    rank_slot0 = []
    for r, nb in enumerate(nb_r):
        rank_slot0.append(len(blocks))
        blocks.extend(range(nb))
    S = len(blocks)
    nch = -(-S // 8)
    Spad = nch * 8
    tot = Spad * 128

    # matmul runs: maximal slot runs with consecutive blocks, same window,
    # within one chunk. (o, n, w, b0, first, last)
    raw = []
    i = 0
    while i < S:
        j = i + 1
        while (j < S and j % 8 != 0 and blocks[j] == blocks[j - 1] + 1
               and blocks[j] // WIN == blocks[i] // WIN):
            j += 1
        raw.append([i, j - i, blocks[i] // WIN, blocks[i], False, False])
        i = j
    firstw = {}
    lastw = {}
    for k, run in enumerate(raw):
        if run[2] not in firstw:
            firstw[run[2]] = k
        lastw[run[2]] = k
    for wdx, k in firstw.items():
        raw[k][4] = True
    for wdx, k in lastw.items():
        raw[k][5] = True
    runs_by_chunk = [[] for _ in range(nch)]
    for (o, n, wdx, b0, fi, la) in raw:
        runs_by_chunk[o // 8].append((o % 8, n, wdx, b0, fi, la))

    # per-rank sds copies: (slot offset, num blocks)
    rank_copies = [(rank_slot0[r], nb_r[r]) for r in range(len(nb_r))]

    per_core = []
    for c in range(C):
        cc = cores[c]
        perm, order, starts, cnt = cc["perm"], cc["order"], cc["starts"], cc["cnt"]
        pi = np.zeros(tot, np.int16)
        par = np.zeros(tot, np.float16)
        wt = np.zeros(tot, np.float16)
        msk = np.zeros(tot, np.float16)
        for r in range(len(nb_r)):
            n = int(cnt[r]) if r < len(cnt) else 0
            if n == 0:
                continue
            o = rank_slot0[r] * 128
            eid = order[starts[perm[:n]] + r]
            sg = _sig(cc["s"][eid])
            pi[o:o + n] = (sg >> 1).astype(np.int16)
            par[o:o + n] = (sg & 1).astype(np.float16)
            wt[o:o + n] = cc["w"][eid].astype(np.float16)
            msk[o:o + n] = 1.0
        sidx = _wrap16(pi)

        def tl(a):
            return np.ascontiguousarray(a.reshape(Spad, 128).T)
        per_core.append(dict(sidx=sidx, par=tl(par), wt=tl(wt), msk=tl(msk)))

    perms = [cores[c]["perm"] for c in range(C)]
    sched = dict(S=S, Spad=Spad, nch=nch, tot=tot, NB=NB, NLP=NLP,
                 runs_by_chunk=tuple(
                     tuple(rc) for rc in runs_by_chunk),
                 rank_copies=tuple(rank_copies))
    return sched, per_core, perms


_BUILD_CACHE = {}


def _build(N, F, O, sched):
    key = (N, F, O, sched["Spad"], sched["runs_by_chunk"],
           sched["rank_copies"], _REP)
    if key in _BUILD_CACHE:
        return _BUILD_CACHE[key]
    NB = sched["NB"]
    NLP = sched["NLP"]
    Spad = sched["Spad"]
    nch = sched["nch"]
    tot = sched["tot"]
    runs_by_chunk = sched["runs_by_chunk"]
    rank_copies = sched["rank_copies"]
    NPAD = -(-N // 256) * 256
    assert NB <= WIN * WIN

    nc = bacc.Bacc("TRN2", target_bir_lowering=False,
                   dynamic_dma_scratch_size=SCR, num_swdge_queues=NQ)
    x_t = nc.dram_tensor("x16", [F, N], F16, kind="ExternalInput")
    w_t = nc.dram_tensor("W", [F, O], F32, kind="ExternalInput")
    a_t = nc.dram_tensor("a", [2 * O], F32, kind="ExternalInput")
    xp_t = nc.dram_tensor("xperm", [F, NLP], F16, kind="ExternalInput")
    sidx_t = nc.dram_tensor("sidx", [128, tot // 16], I16, kind="ExternalInput")
    par_t = nc.dram_tensor("par", [128, Spad], F16, kind="ExternalInput")
    wt_t = nc.dram_tensor("wt", [128, Spad], F16, kind="ExternalInput")
    msk_t = nc.dram_tensor("msk", [128, Spad], F16, kind="ExternalInput")
    out_t = nc.dram_tensor("out", [NLP, O], F32, kind="ExternalOutput")

    with tile.TileContext(nc) as tc:
        with (
            tc.tile_pool(name="persist", bufs=1) as pp,
            tc.tile_pool(name="dram", bufs=1, space="DRAM") as dp,
            tc.tile_pool(name="xpool", bufs=3) as xp,
            tc.tile_pool(name="hpool", bufs=3) as hp_pool,
            tc.tile_pool(name="gpool", bufs=8) as gp,
            tc.tile_pool(name="apool", bufs=3) as ap_,
            tc.tile_pool(name="rpool", bufs=2) as rp,
            tc.tile_pool(name="streams", bufs=2) as sp,
            tc.tile_pool(name="final", bufs=1) as fp,
        ):
            # double-buffered table: rep r+1's phase B writes the other
            # buffer, overlapping rep r's edge-phase gathers
            table0 = dp.tile([NPAD, TROW], F16, tag="tab0")
            if _REP > 1:
                table1 = dp.tile([NPAD, TROW], F16, tag="tab1")
                tables = [table0, table1]
            else:
                tables = [table0]

            # ---- phase A: waug = [W | W@a1] fp16, wa2 fp16 ----
            ident = pp.tile([128, 128], F32)
            make_identity(nc, ident[:])
            idf = pp.tile([128, 128], F16)
            nc.vector.tensor_copy(idf[:], ident[:])
            ws = pp.tile([128, O], F32)
            nc.sync.dma_start(ws[:], w_t[:])
            a1 = pp.tile([O, 1], F32)
            a2 = pp.tile([O, 1], F32)
            nc.sync.dma_start(a1[:], a_t[:O, None])
            nc.sync.dma_start(a2[:], a_t[O:, None])
            waug = pp.tile([128, O + 1], F16)
            wa2f = pp.tile([128, 1], F16)
            with tc.tile_pool(name="psA", bufs=1, space="PSUM") as psa:
                wtp = psa.tile([O, 128], F32, space="PSUM")
                nc.tensor.transpose(out=wtp[:], in_=ws[:], identity=ident[:])
                wts = pp.tile([O, 128], F32)
                nc.vector.tensor_copy(wts[:], wtp[:])
                vab = psa.tile([128, 2], F32, space="PSUM")
                nc.tensor.matmul(out=vab[:, 0:1], lhsT=wts[:], rhs=a1[:],
                                 start=True, stop=True)
                nc.tensor.matmul(out=vab[:, 1:2], lhsT=wts[:], rhs=a2[:],
                                 start=True, stop=True)
                nc.vector.tensor_copy(waug[:, :O], ws[:])
                nc.vector.tensor_copy(waug[:, O:O + 1], vab[:, 0:1])
                nc.vector.tensor_copy(wa2f[:], vab[:, 1:2])

            xpS = sp.tile([128, NLP], F16, tag="xperm")
            nc.sync.dma_start(xpS[:], xp_t[:])

            for rep in range(_REP):
              # ---- streams ----
              sidxS = sp.tile([128, tot // 16], I16, tag="sidx")
              nc.sync.dma_start(sidxS[:], sidx_t[:])
              parS = sp.tile([128, Spad], F16, tag="par")
              nc.sync.dma_start(parS[:], par_t[:])
              wtS = sp.tile([128, Spad], F16, tag="wt")
              nc.sync.dma_start(wtS[:], wt_t[:])
              mskS = sp.tile([128, Spad], F16, tag="msk")
              nc.sync.dma_start(mskS[:], msk_t[:])

              table = tables[rep % len(tables)]
              tpair = table[:].rearrange("(p two) r -> p (two r)", two=2)
              nsc = -(-nch // SK)
              with tc.tile_pool(name=f"psE{rep}", bufs=1, space="PSUM") as pse:
                psacc = pse.tile([128, WIN, 512], F32, space="PSUM", tag="acc")

                # ---- s_dst per position: one-col matmuls into the spare
                # columns of psacc window 6 (blocks only use cols 0..454) ----
                sdpS = sp.tile([128, NB], F16, tag="sdp")
                sdsS = sp.tile([128, Spad], F16, tag="sds")
                if Spad > sched["S"]:
                    nc.vector.memset(sdsS[:, sched["S"]:Spad], 0.0)
                SD0 = WIN * (O + 1)
                assert SD0 + NB <= 512
                for b in range(NB):
                    nc.tensor.matmul(out=psacc[:, WIN - 1, SD0 + b:SD0 + b + 1],
                                     lhsT=xpS[:, b * 128:(b + 1) * 128],
                                     rhs=wa2f[:], start=True, stop=True,
                                     skip_group_check=True)
                nc.scalar.activation(sdpS[:], psacc[:, WIN - 1, SD0:SD0 + NB],
                                     ACTF.Copy)
                for (so, nb) in rank_copies:
                    nc.scalar.activation(sdsS[:, so:so + nb], sdpS[:, :nb],
                                         ACTF.Copy)

                # ---- phase B: table rows [h | s_src | 0pad] fp16 ----
                nbt = -(-NPAD // (XB * 128))
                if "phaseb" in _ABL:
                    nbt = 0
                with tc.tile_pool(name=f"psB{rep}", bufs=1, space="PSUM") as psb:
                  for t in range(nbt):
                    n0 = t * XB * 128
                    nodes = min(XB * 128, NPAD - n0)       # rows this batch
                    nn = max(0, min(nodes, N - n0))        # real nodes
                    ng = nodes // 128
                    xts = xp.tile([F, XB * 128], F16, tag="xts")
                    if nn < nodes:
                        nc.vector.memset(xts[:, nn:nodes], 0.0)
                    nc.sync.dma_start(xts[:, :nn], x_t[:, n0:n0 + nn])
                    hs = hp_pool.tile([128, XB // 2, PAIR], F16, tag="hs")
                    for q in range(0, ng, 4):
                        qn = min(4, ng - q)
                        hpp = psb.tile([128, 4, O + 1], F32, space="PSUM",
                                       tag="hp")
                        for g in range(q, q + qn):
                            nc.tensor.matmul(
                                out=hpp[:, g - q, :],
                                lhsT=xts[:, g * 128:(g + 1) * 128],
                                rhs=waug[:], start=True, stop=True)
                        # parity 0 blocks -> col 0, parity 1 -> col 128
                        nc.scalar.activation(
                            hs[:, q // 2:q // 2 + qn // 2, 0:O + 1],
                            hpp[:, 0:qn:2, :], ACTF.Copy)
                        nc.scalar.activation(
                            hs[:, q // 2:q // 2 + qn // 2, TROW:TROW + O + 1],
                            hpp[:, 1:qn:2, :], ACTF.Copy)
                    nc.sync.dma_start(
                        table[n0:n0 + nodes, :].rearrange(
                            "(g p two) r -> p g (two r)", p=128, two=2),
                        hs[:, :nodes // 256, :])

                # ---- edge phase ----
                for sc in range(nsc):
                    ch0 = sc * SK
                    K = min(SK, nch - ch0)
                    SL = K * 8
                    s0 = ch0 * 8
                    srcS = ap_.tile([128, SK * 8, 2], F16, tag="srcs")
                    gts = []
                    for jj in range(K):
                        j = ch0 + jj
                        gt = gp.tile([128, 8, PAIR], F16, tag="gt")
                        gts.append(gt)
                        if "gather" not in _ABL:
                            nc.gpsimd.dma_gather(
                                out_ap=gt[:, :, :], in_ap=tpair,
                                idxs_ap=sidxS[:, j * GQ // 16:(j + 1) * GQ // 16],
                                num_idxs=GQ, num_idxs_reg=GQ, elem_size=PAIR,
                                queue_num=j % NQ, single_packet=False)
                        elif sc == 0 and jj == 0:
                            nc.vector.memset(gt[:], 0.0)
                        nc.scalar.activation(
                            srcS[:, jj * 8:(jj + 1) * 8, :],
                            gt[:, :, O:PAIR:TROW], ACTF.Copy)
                    # batched alpha on [128, SL]
                    lo = srcS[:, :SL, 0]
                    hi = srcS[:, :SL, 1]
                    pr = parS[:, s0:s0 + SL]
                    ed = ap_.tile([128, SK * 8], F16, tag="ed")
                    e = ap_.tile([128, SK * 8], F16, tag="e")
                    ax = ap_.tile([128, SK * 8], F16, tag="ax")
                    axl = ap_.tile([128, SK * 8], F16, tag="axl")
                    axh = ap_.tile([128, SK * 8], F16, tag="axh")
                    nc.vector.tensor_sub(ed[:, :SL], hi, lo)
                    nc.vector.tensor_mul(ed[:, :SL], ed[:, :SL], pr)
                    nc.vector.tensor_add(e[:, :SL], ed[:, :SL], lo)
                    nc.vector.tensor_add(e[:, :SL], e[:, :SL],
                                         sdsS[:, s0:s0 + SL])
                    nc.vector.tensor_mul(e[:, :SL], e[:, :SL],
                                         wtS[:, s0:s0 + SL])
                    nc.vector.scalar_tensor_tensor(
                        out=ax[:, :SL], in0=e[:, :SL], scalar=0.2,
                        in1=e[:, :SL], op0=ALU.mult, op1=ALU.max)
                    nc.scalar.activation(ax[:, :SL], ax[:, :SL], ACTF.Exp)
                    nc.vector.tensor_mul(ax[:, :SL], ax[:, :SL],
                                         mskS[:, s0:s0 + SL])
                    nc.vector.tensor_mul(axh[:, :SL], ax[:, :SL], pr)
                    nc.vector.tensor_sub(axl[:, :SL], ax[:, :SL], axh[:, :SL])

                    rhsA = rp.tile([128, SK * 8, O + 1], F16, tag="rhsA")
                    rhsB = rp.tile([128, SK * 8, O + 1], F16, tag="rhsB")
                    if "mm" in _ABL:
                        if sc == 0:
                            nc.vector.memset(rhsA[:], 0.0)
                            nc.vector.memset(rhsB[:], 0.0)
                        continue
                    nc.vector.tensor_copy(rhsA[:, :SL, O], axl[:, :SL])
                    nc.vector.tensor_copy(rhsB[:, :SL, O], axh[:, :SL])
                    for jj in range(K):
                        gt = gts[jj]
                        j8 = jj * 8
                        nc.vector.tensor_mul(
                            rhsA[:, j8:j8 + 8, :O], gt[:, :, :O],
                            axl[:, j8:j8 + 8, None].to_broadcast([128, 8, O]))
                        nc.vector.tensor_mul(
                            rhsB[:, j8:j8 + 8, :O], gt[:, :, TROW:TROW + O],
                            axh[:, j8:j8 + 8, None].to_broadcast([128, 8, O]))
                        for (o, n, wdx, b0, fi, la) in runs_by_chunk[ch0 + jj]:
                            oo = jj * 8 + o
                            c0 = (b0 - wdx * WIN) * (O + 1)
                            cn = n * (O + 1)
                            nc.tensor.matmul(
                                out=psacc[:, wdx, c0:c0 + cn],
                                lhsT=idf[:],
                                rhs=rhsA[:, oo:oo + n, :],
                                start=fi, stop=False,
                                skip_group_check=True)
                            nc.tensor.matmul(
                                out=psacc[:, wdx, c0:c0 + cn],
                                lhsT=idf[:],
                                rhs=rhsB[:, oo:oo + n, :],
                                start=False, stop=la,
                                skip_group_check=True)

                # ---- final: out = elu(S / (alpha_sum + 1e-8)) ----
                sacc = fp.tile([128, NB, O + 1], F32, tag="sacc")
                for wdx in range(WIN):
                    nc.scalar.activation(
                        sacc[:, wdx * WIN:(wdx + 1) * WIN, :].rearrange(
                            "p b c -> p (b c)"),
                        psacc[:, wdx, :WIN * (O + 1)], ACTF.Copy)

              rc = fp.tile([128, NB], F32, tag="rc")
              nc.vector.tensor_scalar(out=rc[:], in0=sacc[:, :, O],
                                      scalar1=1e-8, scalar2=None,
                                      op0=ALU.add)
              nc.vector.reciprocal(rc[:], rc[:])
              ov = fp.tile([128, NB, O], F32, tag="ov")
              nc.vector.tensor_mul(ov[:], sacc[:, :, :O],
                                   rc[:, :, None].to_broadcast([128, NB, O]))
              neg = sacc[:, :, :O]
              nc.vector.tensor_scalar(out=neg, in0=ov[:], scalar1=0.0,
                                      scalar2=None, op0=ALU.min)
              nc.scalar.activation(neg, neg, ACTF.Exp)
              nc.vector.tensor_scalar(out=ov[:], in0=ov[:], scalar1=0.0,
                                      scalar2=-1.0, op0=ALU.max,
                                      op1=ALU.add)
              nc.vector.tensor_add(ov[:], ov[:], neg)
              nc.sync.dma_start(
                  out_t[:].rearrange("(b p) f -> p b f", p=128), ov[:])

    nc.compile()
    _BUILD_CACHE[key] = nc
    return nc


def _in_maps(x, W, a, per_core, perms, sched):
    x = np.asarray(x, dtype=np.float32)
    W = np.ascontiguousarray(np.asarray(W, dtype=np.float32))
    a = np.ascontiguousarray(np.asarray(a, dtype=np.float32))
    N, F = x.shape
    NL = N // C
    xT16 = np.ascontiguousarray(x.T.astype(np.float16))
    NLP = sched["NLP"]
    in_maps = []
    for c in range(C):
        pc = per_core[c]
        xperm = np.zeros((F, NLP), np.float16)
        xperm[:, :NL] = xT16[:, c * NL + perms[c]]
        in_maps.append({
            "x16": xT16, "W": W, "a": a, "xperm": xperm,
            "sidx": pc["sidx"], "par": pc["par"], "wt": pc["wt"],
            "msk": pc["msk"],
        })
    return in_maps


def kernel(x, edge_index, edge_weight, W, a):
    x = np.asarray(x, dtype=np.float32)
    W = np.ascontiguousarray(np.asarray(W, dtype=np.float32))
    a = np.ascontiguousarray(np.asarray(a, dtype=np.float32))
    N, F = x.shape
    O = W.shape[1]
    NL = N // C

    sched, per_core, perms = _prep(edge_index, edge_weight, N)
    nc = _build(N, F, O, sched)

    in_maps = _in_maps(x, W, a, per_core, perms, sched)
    res = bass_utils.run_bass_kernel_spmd(nc, in_maps, core_ids=list(range(C)))

    out = np.empty((N, O), np.float32)
    for c in range(C):
        op = res.results[c]["out"]
        out[c * NL + perms[c]] = op[:NL]
    return out


# revision 16
# speedup vs baseline: 1.0994x; 1.0994x over previous
"""GAT layer kernel for 8 Trainium2 NeuronCores — v3.

Edge-parallel dense-rank strategy (dst-sharded) as v2, restructured for
engine balance (v2 was DVE- and phase-B-serialization-bound):

  - pair gathers unchanged (512B elements, GQ=1024, NQ=2 — measured
    optimal at 0.87 ns/idx on HW; 256B elements run at half the rate).
  - alpha math batched over superchunks of 8 chunks (64 slots wide) on
    [128, 64] tiles: ~10 medium DVE ops per 8192 edges instead of ~10
    small ops per 1024 edges. lrelu fused via scalar_tensor_tensor.
  - the hi/lo pair select is folded into the alpha weighting:
    contribution = axl*lo + axh*hi with axl = ax*(1-par), axh = ax*par.
    Two big DVE muls per chunk; no 3-op select chain.
  - accumulation into PSUM-resident [128, 7, 7*65] f32 (7 windows x 7
    blocks x 65 cols = all 49 dst blocks live in PSUM), via identity-lhsT
    matmuls over maximal block-runs; alpha_sum rides along as rhs col 64.
  - phase B: fp16 x input (half the read bytes), 4-block PSUM tiles,
    2 ACT copies per 4 blocks, XB=16 batches, dedicated pools.
  - s_dst per position from a host-permuted fp16 x copy via 49 one-col
    matmuls (replaces v2's phase-B2 table gathers).
"""
import os
import sys
import numpy as np

_REP = int(os.environ.get("KGAT_REPEAT", "1"))
_ABL = set(os.environ.get("KGAT_ABLATE", "").split(","))

try:
    import concourse.bacc as bacc
except ImportError:
    sys.path.insert(0, "/opt/trn_rl_repo")
    import concourse.bacc as bacc
import concourse.tile as tile
import concourse.mybir as mybir
from concourse import bass_utils
from concourse.masks import make_identity

C = 8
GQ = int(os.environ.get("KGAT_GQ", "1024"))
NQ = int(os.environ.get("KGAT_NQ", "2"))
SCR = int(os.environ.get("KGAT_SCR", "36864"))
SK = int(os.environ.get("KGAT_SK", "8"))      # chunks per superchunk
WIN = 7                                        # blocks per PSUM window
TROW = 128            # fp16 per table row (256B)
PAIR = 2 * TROW       # fp16 per gather element (512B)
XB = 16               # phase-B nodes per batch / 128

F32 = mybir.dt.float32
F16 = mybir.dt.float16
I16 = mybir.dt.int16
ALU = mybir.AluOpType
ACTF = mybir.ActivationFunctionType


def _sig(n):
    """Node -> table row: within each 256-group interleave halves so that
    phase B writes row pairs (2p, 2p+1) contiguously from partition p."""
    return (n // 256) * 256 + 2 * (n % 128) + (n % 256) // 128


def _wrap16(a):
    w = np.ascontiguousarray(a.reshape(-1, 16).T)
    return np.tile(w, (8, 1))


def _prep(edge_index, edge_weight, N):
    NL = N // C
    NB = -(-NL // 128)
    NLP = NB * 128
    src = np.asarray(edge_index[0], dtype=np.int64)
    dst = np.asarray(edge_index[1], dtype=np.int64)
    w = np.asarray(edge_weight, dtype=np.float32)

    cores = []
    max_cnt = np.zeros(0, np.int64)
    for c in range(C):
        m = (dst >= c * NL) & (dst < (c + 1) * NL)
        s_c = src[m]
        d_c = dst[m] - c * NL
        w_c = w[m]
        deg = np.bincount(d_c, minlength=NL)
        perm = np.argsort(-deg, kind="stable")
        order = np.argsort(d_c, kind="stable")
        starts = np.zeros(NL + 1, np.int64)
        starts[1:] = np.cumsum(deg)
        maxdeg = int(deg.max()) if deg.size else 0
        hist = np.bincount(deg, minlength=maxdeg + 2)
        cnt = NL - np.cumsum(hist)[:maxdeg + 1]
        cnt = cnt[cnt > 0]
        cores.append(dict(s=s_c, w=w_c, perm=perm, order=order,
                          starts=starts, cnt=cnt))
        if len(cnt) > len(max_cnt):
            mc = np.zeros(len(cnt), np.int64)
            mc[:len(max_cnt)] = max_cnt
            max_cnt = mc
        max_cnt[:len(cnt)] = np.maximum(max_cnt[:len(cnt)], cnt)

    # rank 0 must cover every position block so its runs span full PSUM
    # windows (start=True resets whole windows)
    max_cnt[0] = NLP

    # unified slot stream: rank-major, block = slot index within rank
    nb_r = [int(-(-int(x) // 128)) for x in max_cnt]
    blocks = []
    rank_slot0 = []
    for r, nb in enumerate(nb_r):
        rank_slot0.append(len(blocks))
        blocks.extend(range(nb))
    S = len(blocks)
    nch = -(-S // 8)
    Spad = nch * 8
    tot = Spad * 128

    # matmul runs: maximal slot runs with consecutive blocks, same window,
    # within one chunk. (o, n, w, b0, first, last)
    raw = []
    i = 0
    while i < S:
        j = i + 1
        while (j < S and j % 8 != 0 and blocks[j] == blocks[j - 1] + 1
               and blocks[j] // WIN == blocks[i] // WIN):
            j += 1
        raw.append([i, j - i, blocks[i] // WIN, blocks[i], False, False])
        i = j
    firstw = {}
    lastw = {}
    for k, run in enumerate(raw):
        if run[2] not in firstw:
            firstw[run[2]] = k
        lastw[run[2]] = k
    for wdx, k in firstw.items():
        raw[k][4] = True
    for wdx, k in lastw.items():
        raw[k][5] = True
    runs_by_chunk = [[] for _ in range(nch)]
    for (o, n, wdx, b0, fi, la) in raw:
        runs_by_chunk[o // 8].append((o % 8, n, wdx, b0, fi, la))

    # per-rank sds copies: (slot offset, num blocks)
    rank_copies = [(rank_slot0[r], nb_r[r]) for r in range(len(nb_r))]

    per_core = []
    for c in range(C):
        cc = cores[c]
        perm, order, starts, cnt = cc["perm"], cc["order"], cc["starts"], cc["cnt"]
        pi = np.zeros(tot, np.int16)
        par = np.zeros(tot, np.float16)
        wt = np.zeros(tot, np.float16)
        msk = np.zeros(tot, np.float16)
        for r in range(len(nb_r)):
            n = int(cnt[r]) if r < len(cnt) else 0
            if n == 0:
                continue
            o = rank_slot0[r] * 128
            eid = order[starts[perm[:n]] + r]
            sg = _sig(cc["s"][eid])
            pi[o:o + n] = (sg >> 1).astype(np.int16)
            par[o:o + n] = (sg & 1).astype(np.float16)
            wt[o:o + n] = cc["w"][eid].astype(np.float16)
            msk[o:o + n] = 1.0
        sidx = _wrap16(pi)

        def tl(a):
            return np.ascontiguousarray(a.reshape(Spad, 128).T)
        per_core.append(dict(sidx=sidx, par=tl(par), wt=tl(wt), msk=tl(msk)))

    perms = [cores[c]["perm"] for c in range(C)]
    sched = dict(S=S, Spad=Spad, nch=nch, tot=tot, NB=NB, NLP=NLP,
                 runs_by_chunk=tuple(
                     tuple(rc) for rc in runs_by_chunk),
                 rank_copies=tuple(rank_copies))
    return sched, per_core, perms


_BUILD_CACHE = {}


def _build(N, F, O, sched):
    key = (N, F, O, sched["Spad"], sched["runs_by_chunk"],
           sched["rank_copies"], _REP)
    if key in _BUILD_CACHE:
        return _BUILD_CACHE[key]
    NB = sched["NB"]
    NLP = sched["NLP"]
    Spad = sched["Spad"]
    nch = sched["nch"]
    tot = sched["tot"]
    runs_by_chunk = sched["runs_by_chunk"]
    rank_copies = sched["rank_copies"]
    NPAD = -(-N // 256) * 256
    assert NB <= WIN * WIN

    nc = bacc.Bacc("TRN2", target_bir_lowering=False,
                   dynamic_dma_scratch_size=SCR, num_swdge_queues=NQ)
    x_t = nc.dram_tensor("x16", [F, N], F16, kind="ExternalInput")
    w_t = nc.dram_tensor("W", [F, O], F32, kind="ExternalInput")
    a_t = nc.dram_tensor("a", [2 * O], F32, kind="ExternalInput")
    xp_t = nc.dram_tensor("xperm", [F, NLP], F16, kind="ExternalInput")
    sidx_t = nc.dram_tensor("sidx", [128, tot // 16], I16, kind="ExternalInput")
    par_t = nc.dram_tensor("par", [128, Spad], F16, kind="ExternalInput")
    wt_t = nc.dram_tensor("wt", [128, Spad], F16, kind="ExternalInput")
    msk_t = nc.dram_tensor("msk", [128, Spad], F16, kind="ExternalInput")
    out_t = nc.dram_tensor("out", [NLP, O], F32, kind="ExternalOutput")

    with tile.TileContext(nc) as tc:
        with (
            tc.tile_pool(name="persist", bufs=1) as pp,
            tc.tile_pool(name="dram", bufs=1, space="DRAM") as dp,
            tc.tile_pool(name="xpool", bufs=3) as xp,
            tc.tile_pool(name="hpool", bufs=3) as hp_pool,
            tc.tile_pool(name="gpool", bufs=8) as gp,
            tc.tile_pool(name="apool", bufs=3) as ap_,
            tc.tile_pool(name="rpool", bufs=2) as rp,
            tc.tile_pool(name="streams", bufs=2) as sp,
            tc.tile_pool(name="final", bufs=1) as fp,
        ):
            # double-buffered table: rep r+1's phase B writes the other
            # buffer, overlapping rep r's edge-phase gathers
            table0 = dp.tile([NPAD, TROW], F16, tag="tab0")
            if _REP > 1:
                table1 = dp.tile([NPAD, TROW], F16, tag="tab1")
                tables = [table0, table1]
            else:
                tables = [table0]

            # ---- phase A: waug = [W | W@a1] fp16, wa2 fp16 ----
            ident = pp.tile([128, 128], F32)
            make_identity(nc, ident[:])
            idf = pp.tile([128, 128], F16)
            nc.vector.tensor_copy(idf[:], ident[:])
            ws = pp.tile([128, O], F32)
            nc.sync.dma_start(ws[:], w_t[:])
            a1 = pp.tile([O, 1], F32)
            a2 = pp.tile([O, 1], F32)
            nc.sync.dma_start(a1[:], a_t[:O, None])
            nc.sync.dma_start(a2[:], a_t[O:, None])
            waug = pp.tile([128, O + 1], F16)
            wa2f = pp.tile([128, 1], F16)
            with tc.tile_pool(name="psA", bufs=1, space="PSUM") as psa:
                wtp = psa.tile([O, 128], F32, space="PSUM")
                nc.tensor.transpose(out=wtp[:], in_=ws[:], identity=ident[:])
                wts = pp.tile([O, 128], F32)
                nc.vector.tensor_copy(wts[:], wtp[:])
                vab = psa.tile([128, 2], F32, space="PSUM")
                nc.tensor.matmul(out=vab[:, 0:1], lhsT=wts[:], rhs=a1[:],
                                 start=True, stop=True)
                nc.tensor.matmul(out=vab[:, 1:2], lhsT=wts[:], rhs=a2[:],
                                 start=True, stop=True)
                nc.vector.tensor_copy(waug[:, :O], ws[:])
                nc.vector.tensor_copy(waug[:, O:O + 1], vab[:, 0:1])
                nc.vector.tensor_copy(wa2f[:], vab[:, 1:2])

            xpS = sp.tile([128, NLP], F16, tag="xperm")
            nc.sync.dma_start(xpS[:], xp_t[:])

            def emit_streams(rep):
                sidxS = sp.tile([128, tot // 16], I16, tag="sidx")
                nc.sync.dma_start(sidxS[:], sidx_t[:])
                parS = sp.tile([128, Spad], F16, tag="par")
                nc.sync.dma_start(parS[:], par_t[:])
                wtS = sp.tile([128, Spad], F16, tag="wt")
                nc.sync.dma_start(wtS[:], wt_t[:])
                mskS = sp.tile([128, Spad], F16, tag="msk")
                nc.sync.dma_start(mskS[:], msk_t[:])
                return sidxS, parS, wtS, mskS

            nbt = 0 if "phaseb" in _ABL else -(-NPAD // (XB * 128))

            def emit_batch(psb, table, t):
                n0 = t * XB * 128
                nodes = min(XB * 128, NPAD - n0)       # rows this batch
                nn = max(0, min(nodes, N - n0))        # real nodes
                ng = nodes // 128
                xts = xp.tile([F, XB * 128], F16, tag="xts")
                if nn < nodes:
                    nc.vector.memset(xts[:, nn:nodes], 0.0)
                nc.sync.dma_start(xts[:, :nn], x_t[:, n0:n0 + nn])
                hs = hp_pool.tile([128, XB // 2, PAIR], F16, tag="hs")
                for q in range(0, ng, 4):
                    qn = min(4, ng - q)
                    hpp = psb.tile([128, 4, O + 1], F32, space="PSUM",
                                   tag="hp")
                    for g in range(q, q + qn):
                        nc.tensor.matmul(
                            out=hpp[:, g - q, :],
                            lhsT=xts[:, g * 128:(g + 1) * 128],
                            rhs=waug[:], start=True, stop=True)
                    # parity 0 blocks -> col 0, parity 1 -> col 128
                    nc.scalar.activation(
                        hs[:, q // 2:q // 2 + qn // 2, 0:O + 1],
                        hpp[:, 0:qn:2, :], ACTF.Copy)
                    nc.scalar.activation(
                        hs[:, q // 2:q // 2 + qn // 2, TROW:TROW + O + 1],
                        hpp[:, 1:qn:2, :], ACTF.Copy)
                nc.sync.dma_start(
                    table[n0:n0 + nodes, :].rearrange(
                        "(g p two) r -> p g (two r)", p=128, two=2),
                    hs[:, :nodes // 256, :])

            STR = {0: emit_streams(0)}
            with tc.tile_pool(name="psB0", bufs=1, space="PSUM") as psb0:
                for t in range(nbt):
                    emit_batch(psb0, tables[0], t)
            nsc = -(-nch // SK)
            for rep in range(_REP):
              sidxS, parS, wtS, mskS = STR[rep]
              table = tables[rep % len(tables)]
              tpair = table[:].rearrange("(p two) r -> p (two r)", two=2)
              with tc.tile_pool(name=f"psE{rep}", bufs=1, space="PSUM") as pse:
                psacc = pse.tile([128, WIN, 512], F32, space="PSUM", tag="acc")

                # ---- s_dst per position: one-col matmuls into the spare
                # columns of psacc window 6 (blocks only use cols 0..454) ----
                sdpS = sp.tile([128, NB], F16, tag="sdp")
                sdsS = sp.tile([128, Spad], F16, tag="sds")
                if Spad > sched["S"]:
                    nc.vector.memset(sdsS[:, sched["S"]:Spad], 0.0)
                SD0 = WIN * (O + 1)
                assert SD0 + NB <= 512
                for b in range(NB):
                    nc.tensor.matmul(out=psacc[:, WIN - 1, SD0 + b:SD0 + b + 1],
                                     lhsT=xpS[:, b * 128:(b + 1) * 128],
                                     rhs=wa2f[:], start=True, stop=True,
                                     skip_group_check=True)
                nc.scalar.activation(sdpS[:], psacc[:, WIN - 1, SD0:SD0 + NB],
                                     ACTF.Copy)
                for (so, nb) in rank_copies:
                    nc.scalar.activation(sdsS[:, so:so + nb], sdpS[:, :nb],
                                         ACTF.Copy)

                # next rep's streams + phase B, interleaved into this rep's
                # edge phase so the in-order engine streams overlap them
                nxt = rep + 1
                nxt_batches = []
                psb_cm = psb_nxt = None
                if nxt < _REP:
                    STR[nxt] = emit_streams(nxt)
                    psb_cm = tc.tile_pool(name=f"psB{nxt}", bufs=1,
                                          space="PSUM")
                    psb_nxt = psb_cm.__enter__()
                    nxt_batches = list(range(nbt))

                # ---- edge phase ----
                for sc in range(nsc):
                    while nxt_batches and len(nxt_batches) > (
                            nbt * (nsc - 1 - sc)) // nsc:
                        emit_batch(psb_nxt, tables[nxt % len(tables)],
                                   nxt_batches.pop(0))
                    ch0 = sc * SK
                    K = min(SK, nch - ch0)
                    SL = K * 8
                    s0 = ch0 * 8
                    srcS = ap_.tile([128, SK * 8, 2], F16, tag="srcs")
                    gts = []
                    for jj in range(K):
                        j = ch0 + jj
                        gt = gp.tile([128, 8, PAIR], F16, tag="gt")
                        gts.append(gt)
                        if "gather" not in _ABL:
                            nc.gpsimd.dma_gather(
                                out_ap=gt[:, :, :], in_ap=tpair,
                                idxs_ap=sidxS[:, j * GQ // 16:(j + 1) * GQ // 16],
                                num_idxs=GQ, num_idxs_reg=GQ, elem_size=PAIR,
                                queue_num=j % NQ, single_packet=False)
                        elif sc == 0 and jj == 0:
                            nc.vector.memset(gt[:], 0.0)
                        nc.scalar.activation(
                            srcS[:, jj * 8:(jj + 1) * 8, :],
                            gt[:, :, O:PAIR:TROW], ACTF.Copy)
                    # batched alpha on [128, SL]
                    lo = srcS[:, :SL, 0]
                    hi = srcS[:, :SL, 1]
                    pr = parS[:, s0:s0 + SL]
                    ed = ap_.tile([128, SK * 8], F16, tag="ed")
                    e = ap_.tile([128, SK * 8], F16, tag="e")
                    ax = ap_.tile([128, SK * 8], F16, tag="ax")
                    axl = ap_.tile([128, SK * 8], F16, tag="axl")
                    axh = ap_.tile([128, SK * 8], F16, tag="axh")
                    nc.vector.tensor_sub(ed[:, :SL], hi, lo)
                    nc.vector.tensor_mul(ed[:, :SL], ed[:, :SL], pr)
                    nc.vector.tensor_add(e[:, :SL], ed[:, :SL], lo)
                    nc.vector.tensor_add(e[:, :SL], e[:, :SL],
                                         sdsS[:, s0:s0 + SL])
                    nc.vector.tensor_mul(e[:, :SL], e[:, :SL],
                                         wtS[:, s0:s0 + SL])
                    nc.vector.scalar_tensor_tensor(
                        out=ax[:, :SL], in0=e[:, :SL], scalar=0.2,
                        in1=e[:, :SL], op0=ALU.mult, op1=ALU.max)
                    nc.scalar.activation(ax[:, :SL], ax[:, :SL], ACTF.Exp)
                    nc.vector.tensor_mul(ax[:, :SL], ax[:, :SL],
                                         mskS[:, s0:s0 + SL])
                    nc.vector.tensor_mul(axh[:, :SL], ax[:, :SL], pr)
                    nc.vector.tensor_sub(axl[:, :SL], ax[:, :SL], axh[:, :SL])

                    rhsA = rp.tile([128, SK * 8, O + 1], F16, tag="rhsA")
                    rhsB = rp.tile([128, SK * 8, O + 1], F16, tag="rhsB")
                    if "mm" in _ABL:
                        if sc == 0:
                            nc.vector.memset(rhsA[:], 0.0)
                            nc.vector.memset(rhsB[:], 0.0)
                        continue
                    nc.vector.tensor_copy(rhsA[:, :SL, O], axl[:, :SL])
                    nc.vector.tensor_copy(rhsB[:, :SL, O], axh[:, :SL])
                    for jj in range(K):
                        gt = gts[jj]
                        j8 = jj * 8
                        nc.vector.tensor_mul(
                            rhsA[:, j8:j8 + 8, :O], gt[:, :, :O],
                            axl[:, j8:j8 + 8, None].to_broadcast([128, 8, O]))
                        nc.vector.tensor_mul(
                            rhsB[:, j8:j8 + 8, :O], gt[:, :, TROW:TROW + O],
                            axh[:, j8:j8 + 8, None].to_broadcast([128, 8, O]))
                        for (o, n, wdx, b0, fi, la) in runs_by_chunk[ch0 + jj]:
                            oo = jj * 8 + o
                            c0 = (b0 - wdx * WIN) * (O + 1)
                            cn = n * (O + 1)
                            nc.tensor.matmul(
                                out=psacc[:, wdx, c0:c0 + cn],
                                lhsT=idf[:],
                                rhs=rhsA[:, oo:oo + n, :],
                                start=fi, stop=False,
                                skip_group_check=True)
                            nc.tensor.matmul(
                                out=psacc[:, wdx, c0:c0 + cn],
                                lhsT=idf[:],
                                rhs=rhsB[:, oo:oo + n, :],
                                start=False, stop=la,
                                skip_group_check=True)

                if psb_cm is not None:
                    psb_cm.__exit__(None, None, None)

                # ---- final: out = elu(S / (alpha_sum + 1e-8)) ----
                sacc = fp.tile([128, NB, O + 1], F32, tag="sacc")
                for wdx in range(WIN):
                    nc.scalar.activation(
                        sacc[:, wdx * WIN:(wdx + 1) * WIN, :].rearrange(
                            "p b c -> p (b c)"),
                        psacc[:, wdx, :WIN * (O + 1)], ACTF.Copy)

              rc = fp.tile([128, NB], F32, tag="rc")
              nc.vector.tensor_scalar(out=rc[:], in0=sacc[:, :, O],
                                      scalar1=1e-8, scalar2=None,
                                      op0=ALU.add)
              nc.vector.reciprocal(rc[:], rc[:])
              ov = fp.tile([128, NB, O], F32, tag="ov")
              nc.vector.tensor_mul(ov[:], sacc[:, :, :O],
                                   rc[:, :, None].to_broadcast([128, NB, O]))
              neg = sacc[:, :, :O]
              nc.vector.tensor_scalar(out=neg, in0=ov[:], scalar1=0.0,
                                      scalar2=None, op0=ALU.min)
              nc.scalar.activation(neg, neg, ACTF.Exp)
              nc.vector.tensor_scalar(out=ov[:], in0=ov[:], scalar1=0.0,
                                      scalar2=-1.0, op0=ALU.max,
                                      op1=ALU.add)
              nc.vector.tensor_add(ov[:], ov[:], neg)
              nc.sync.dma_start(
                  out_t[:].rearrange("(b p) f -> p b f", p=128), ov[:])

    nc.compile()
    _BUILD_CACHE[key] = nc
    return nc


def _in_maps(x, W, a, per_core, perms, sched):
    x = np.asarray(x, dtype=np.float32)
    W = np.ascontiguousarray(np.asarray(W, dtype=np.float32))
    a = np.ascontiguousarray(np.asarray(a, dtype=np.float32))
    N, F = x.shape
    NL = N // C
    xT16 = np.ascontiguousarray(x.T.astype(np.float16))
    NLP = sched["NLP"]
    in_maps = []
    for c in range(C):
        pc = per_core[c]
        xperm = np.zeros((F, NLP), np.float16)
        xperm[:, :NL] = xT16[:, c * NL + perms[c]]
        in_maps.append({
            "x16": xT16, "W": W, "a": a, "xperm": xperm,
            "sidx": pc["sidx"], "par": pc["par"], "wt": pc["wt"],
            "msk": pc["msk"],
        })
    return in_maps


def kernel(x, edge_index, edge_weight, W, a):
    x = np.asarray(x, dtype=np.float32)
    W = np.ascontiguousarray(np.asarray(W, dtype=np.float32))
    a = np.ascontiguousarray(np.asarray(a, dtype=np.float32))
    N, F = x.shape
    O = W.shape[1]
    NL = N // C

    sched, per_core, perms = _prep(edge_index, edge_weight, N)
    nc = _build(N, F, O, sched)

    in_maps = _in_maps(x, W, a, per_core, perms, sched)
    res = bass_utils.run_bass_kernel_spmd(nc, in_maps, core_ids=list(range(C)))

    out = np.empty((N, O), np.float32)
    for c in range(C):
        op = res.results[c]["out"]
        out[c * NL + perms[c]] = op[:NL]
    return out
